# revision 4
# baseline (speedup 1.0000x reference)
"""Trainium2 Bass kernel for nn_CubeMoveHead.

Contract: kernel(**inputs) takes the FULL unsharded inputs (as produced by
setup_inputs) and returns the FULL [512, 1536] float32 output.

Strategy (data-parallel over graphs, 64 graphs per core on 8 cores):
  Only the first 64 cube nodes of each graph ever reach the output, so the
  host computes those node indices (pure index math on cube_mask/batch),
  gathers just the needed node_features rows (4096 per core), transposes
  them to the matmul-friendly [D, nodes] layout, and ships them to each
  core's HBM in bf16. Nodes are laid out slot-major (node j on a core is
  cube slot c = j // 64 of graph g = j % 64), so the per-graph global
  feature column tiles periodically: gf_rep[:, j] = gf[j % 64].

  All matmul inputs are bf16 (f32 PSUM accumulate): measured end-to-end
  rel err ~4e-3 against the f32 reference, well inside the 2e-2 gate.

  On-device per core, for each of 8 tiles of 512 node slots:
    ps  = W1a.T @ x_t + W1b.T @ gf_rep   (two accumulating matmuls, PSUM)
    h   = relu(ps)  -> bf16              (ACT, PSUM->SBUF)
    ps2 = W2.T @ h                       ([24, 512] PSUM; W2 stationary so
                                          the whole tile streams in one
                                          512-col matmul instead of 4
                                          LDWEIGHTS+24-col matmuls)
    o   = min(ps2, cap)                  (DVE; cap = +BIG where slot valid
                                          & move allowed, else NEG)
  min-cap masking yields exactly NEG on masked positions; it folds b1/b2,
  which are identically zero in the reference (asserted host-side).

  The first matmuls start right after the small weight DMA lands and run
  back-to-back with no long PE gaps, so the HAM clock gate warms to 2.4
  GHz early and stays there (the previous version stalled the PE ~4us
  waiting for a late DMA and ran the whole kernel re-throttled at 1.2).
"""

import sys

if "/opt/trn_rl_repo" not in sys.path:
    sys.path.insert(0, "/opt/trn_rl_repo")

import ml_dtypes
import numpy as np

import concourse.bass as bass
import concourse.mybir as mybir
from concourse.tile import TileContext
from concourse.bass_utils import run_bass_kernel_spmd

N = 500000
B = 512
D = 128
G = 128
MC = 64
M = 24
H = 128
NEG = -1.0e9
BIG = 3.0e38
NCORES = 8
GPC = B // NCORES          # graphs per core (64)
S = GPC * MC               # node slots per core (4096)
NT = S // 512              # 512-slot tiles per core (8)


def _legalize_single_wait(nc):
    """The walrus build here accepts at most ONE sync wait per instruction;
    Tile's scheduler happily emits several. Hoist extra waits onto same-engine
    nops inserted immediately before the offending instruction (same engine
    executes in order, so the happens-before is preserved exactly)."""
    for f in nc.m.functions:
        for bb in f.blocks:
            insts = bb.instructions
            if not any(
                i.sync_info and i.sync_info.on_wait and len(i.sync_info.on_wait) > 1
                for i in insts
            ):
                continue
            out = []
            for inst in insts:
                si = inst.sync_info
                waits = list(si.on_wait) if si and si.on_wait else []
                if len(waits) > 1:
                    for w in waits[:-1]:
                        nop = mybir.InstNoOp(
                            name=nc.get_next_instruction_name(), ins=[], outs=[]
                        )
                        nop.engine = inst.engine
                        nop.sync_info = mybir.SyncInfo(on_wait=[w], on_update=[])
                        nop.bass_nofuse = True
                        nc.register_instruction(nop)
                        out.append(nop)
                    si.on_wait = [waits[-1]]
                out.append(inst)
            bb.instructions[:] = out


def _build_program():
    f32 = mybir.dt.float32
    bf16 = mybir.dt.bfloat16
    nc = bass.Bass()
    x_d = nc.declare_dram_parameter("x", [D, S], bf16, isOutput=False)
    # wg packs all small bf16 constants: W1a | W1b | W2 | gf_rep
    WGW = 2 * H + M + 512
    wg_d = nc.declare_dram_parameter("wg", [128, WGW], bf16, isOutput=False)
    cap_d = nc.declare_dram_parameter("cap", [M, S], f32, isOutput=False)
    o_d = nc.declare_dram_parameter("o", [M, S], f32, isOutput=True)

    relu = mybir.ActivationFunctionType.Relu

    with TileContext(nc) as tc:
        with (
            tc.tile_pool(name="consts", bufs=1) as cpool,
            tc.tile_pool(name="x", bufs=NT // 2) as xpool,
            tc.tile_pool(name="h", bufs=3) as hpool,
            tc.tile_pool(name="ps", bufs=3, space="PSUM") as pspool,
            tc.tile_pool(name="pswarm", bufs=1, space="PSUM") as pswpool,
            tc.tile_pool(name="ps2", bufs=2, space="PSUM") as ps2pool,
            tc.tile_pool(name="o", bufs=1) as opool,
            tc.tile_pool(name="scratch", bufs=1) as spool,
        ):
            # Warmups, fed by an on-chip memset (no DMA dependency): a
            # [128,1] relu so ACT's PWP table loads during the DMA wait, and
            # two short bf16 matmuls that keep the PE's HAM activity window
            # busy during the x DMA wait (cheap — unlike f32 warmups, which
            # stream at 1/4 rate and monopolize the PE for ~4us).
            warm = spool.tile([128, 512], bf16)
            nc.vector.memset(warm[:], 0.0)
            nc.scalar.activation(warm[:, 0:1], warm[:, 0:1], relu)
            pswarm = pswpool.tile([128, 512], f32)
            for _ in range(2):
                nc.tensor.matmul(
                    pswarm[:], warm[:, 0:128], warm[:], start=True, stop=True
                )

            wg_sb = cpool.tile([128, WGW], bf16)
            nc.sync.dma_start(out=wg_sb[:], in_=wg_d[:])
            w1a_sb = wg_sb[:, 0:H]
            w1b_sb = wg_sb[:, H:2 * H]
            w2_sb = wg_sb[:, 2 * H:2 * H + M]
            gfr_sb = wg_sb[:, 2 * H + M:WGW]

            xts = [
                xpool.tile([D, 1024], bf16, name=f"xt{i}", tag=f"x{i}")
                for i in range(NT // 2)
            ]
            nc.sync.dma_start(out=xts[0][:], in_=x_d[:, 0:1024])
            cap_sb = cpool.tile([M, S], f32)
            nc.gpsimd.dma_start(out=cap_sb[:], in_=cap_d[:])
            for c in range(1, NT // 2):
                nc.sync.dma_start(
                    out=xts[c][:], in_=x_d[:, c * 1024:(c + 1) * 1024]
                )

            o_sb = opool.tile([M, S], f32)

            for t in range(NT):
                xt = xts[t // 2][:, (t % 2) * 512:(t % 2 + 1) * 512]
                ps = pspool.tile([128, 512], f32)
                nc.tensor.matmul(ps[:], w1a_sb, xt, start=True, stop=False)
                nc.tensor.matmul(ps[:], w1b_sb, gfr_sb, start=False, stop=True)
                h = hpool.tile([128, 512], bf16)
                nc.scalar.activation(h[:], ps[:], relu)
                ps2 = ps2pool.tile([M, 512], f32)
                nc.tensor.matmul(ps2[:], w2_sb, h[:], start=True, stop=True)
                # out = min(ps2, cap): exact NEG on masked slots (b1 == b2 == 0).
                # NOTE: scalar_tensor_tensor hangs the HW here; tensor_tensor
                # with op=min is the verified-working form.
                nc.vector.tensor_tensor(
                    o_sb[:, t * 512:(t + 1) * 512],
                    ps2[:],
                    cap_sb[:, t * 512:(t + 1) * 512],
                    op=mybir.AluOpType.min,
                )
                if t == NT // 2 - 1:
                    nc.gpsimd.dma_start(
                        out=o_d[:, :S // 2], in_=o_sb[:, :S // 2]
                    )
            nc.gpsimd.dma_start(out=o_d[:, S // 2:], in_=o_sb[:, S // 2:])
    _legalize_single_wait(nc)
    return nc


_NC_CACHE = None


def _get_program():
    global _NC_CACHE
    if _NC_CACHE is None:
        _NC_CACHE = _build_program()
    return _NC_CACHE


def _prepare_inputs(node_features, global_features, W1, b1, W2, b2, cube_mask,
                    batch, move_mask):
    """Host-side shard prep. Returns per-core input dicts."""
    node_features = np.asarray(node_features, dtype=np.float32)
    global_features = np.asarray(global_features, dtype=np.float32)
    W1 = np.asarray(W1, dtype=np.float32)
    b1 = np.asarray(b1, dtype=np.float32)
    W2 = np.asarray(W2, dtype=np.float32)
    b2 = np.asarray(b2, dtype=np.float32)
    cube_mask = np.asarray(cube_mask).astype(bool)
    batch = np.asarray(batch).astype(np.int64)
    move_mask = np.asarray(move_mask).astype(bool)
    assert np.all(b1 == 0.0) and np.all(b2 == 0.0), (
        "kernel bakes b1==b2==0 into the min-cap masking"
    )

    # First-64 cube nodes per graph (matches the reference's cube_idx math).
    idx = np.flatnonzero(cube_mask)                     # cube nodes, node order
    cb = batch[idx]                                     # their graph (sorted)
    counts = np.bincount(cb, minlength=B)
    starts = np.concatenate([[0], np.cumsum(counts)[:-1]])
    pos = np.arange(idx.shape[0], dtype=np.int64) - starts[cb]
    sel = pos < MC
    vidx, vb, vpos = idx[sel], cb[sel], pos[sel]

    gather_idx = np.zeros((B, MC), dtype=np.int64)
    valid = np.zeros((B, MC), dtype=bool)
    gather_idx[vb, vpos] = vidx
    valid[vb, vpos] = True

    wcat = np.concatenate([W1[:D], W1[D:], W2], axis=1)  # [128, 2H + M]

    in_maps = []
    for k in range(NCORES):
        gb = slice(k * GPC, (k + 1) * GPC)
        gi = gather_idx[gb]                             # [GPC, MC]
        # slot-major: node j = c*GPC + g  ->  (cube slot c, graph g)
        order = gi.T.reshape(-1)                        # [S]
        x = np.ascontiguousarray(
            node_features[order].T.astype(ml_dtypes.bfloat16)
        )                                               # [D, S]
        gfr = np.tile(global_features[gb].T, (1, 512 // GPC))    # [G, 512]
        wg = np.ascontiguousarray(
            np.concatenate([wcat, gfr], axis=1).astype(ml_dtypes.bfloat16)
        )                                               # [128, 2H + M + 512]
        ok = valid[gb].T.reshape(-1)[:, None] & \
            move_mask[gb].transpose(1, 0, 2).reshape(S, M)       # [S, M]
        cap = np.ascontiguousarray(
            np.where(ok.T, np.float32(BIG), np.float32(NEG)).astype(np.float32)
        )                                               # [M, S]
        in_maps.append({"x": x, "wg": wg, "cap": cap})
    return in_maps


def _decode_outputs(results):
    logits = np.empty((B, MC, M), dtype=np.float32)
    for k in range(NCORES):
        o = results[k]["o"]                              # [M, S]
        # slot-major: column j = c*GPC + g
        logits[k * GPC:(k + 1) * GPC] = o.reshape(M, MC, GPC).transpose(2, 1, 0)
    return logits.reshape(B, MC * M)


def kernel(**inputs) -> np.ndarray:
    in_maps = _prepare_inputs(**inputs)
    nc = _get_program()
    res = run_bass_kernel_spmd(nc, in_maps, list(range(NCORES)))
    return _decode_outputs(res.results)


# revision 7
# speedup vs baseline: 1.0822x; 1.0822x over previous
"""Trainium2 Bass kernel for nn_CubeMoveHead.

Contract: kernel(**inputs) takes the FULL unsharded inputs (as produced by
setup_inputs) and returns the FULL [512, 1536] float32 output.

Strategy (data-parallel over graphs, 64 graphs per core on 8 cores):
  Only the first 64 cube nodes of each graph ever reach the output, so the
  host computes those node indices (pure index math on cube_mask/batch),
  gathers just the needed node_features rows (4096 per core), transposes
  them to the matmul-friendly [D, nodes] layout, and ships them to each
  core's HBM in bf16. Nodes are laid out slot-major (node j on a core is
  cube slot c = j // 64 of graph g = j % 64), so the per-graph global
  feature column tiles periodically: gf_rep[:, j] = gf[j % 64].

  All matmul inputs are bf16 (f32 PSUM accumulate): measured end-to-end
  rel err ~4e-3 against the f32 reference, well inside the 2e-2 gate.

  On-device per core, for each of 8 tiles of 512 node slots:
    ps  = W1a.T @ x_t + W1b.T @ gf_rep   (two accumulating matmuls, PSUM)
    h   = relu(ps)  -> bf16              (ACT, PSUM->SBUF)
    ps2 = W2.T @ h                       ([24, 512] PSUM; W2 stationary so
                                          the whole tile streams in one
                                          512-col matmul instead of 4
                                          LDWEIGHTS+24-col matmuls)
    o   = min(ps2, cap)                  (DVE; cap = +BIG where slot valid
                                          & move allowed, else NEG)
  min-cap masking yields exactly NEG on masked positions; it folds b1/b2,
  which are identically zero in the reference (asserted host-side).

  The first matmuls start right after the small weight DMA lands and run
  back-to-back with no long PE gaps, so the HAM clock gate warms to 2.4
  GHz early and stays there (the previous version stalled the PE ~4us
  waiting for a late DMA and ran the whole kernel re-throttled at 1.2).
"""

import sys

if "/opt/trn_rl_repo" not in sys.path:
    sys.path.insert(0, "/opt/trn_rl_repo")

import ml_dtypes
import numpy as np

import concourse.bass as bass
import concourse.mybir as mybir
from concourse.tile import TileContext
from concourse.bass_utils import run_bass_kernel_spmd

N = 500000
B = 512
D = 128
G = 128
MC = 64
M = 24
H = 128
NEG = -1.0e9
BIG = 3.0e38
NCORES = 8
GPC = B // NCORES          # graphs per core (64)
S = GPC * MC               # node slots per core (4096)
NT = S // 512              # 512-slot tiles per core (8)


def _legalize_single_wait(nc):
    """The walrus build here accepts at most ONE sync wait per instruction;
    Tile's scheduler happily emits several. Hoist extra waits onto same-engine
    nops inserted immediately before the offending instruction (same engine
    executes in order, so the happens-before is preserved exactly)."""
    for f in nc.m.functions:
        for bb in f.blocks:
            insts = bb.instructions
            if not any(
                i.sync_info and i.sync_info.on_wait and len(i.sync_info.on_wait) > 1
                for i in insts
            ):
                continue
            out = []
            for inst in insts:
                si = inst.sync_info
                waits = list(si.on_wait) if si and si.on_wait else []
                if len(waits) > 1:
                    for w in waits[:-1]:
                        nop = mybir.InstNoOp(
                            name=nc.get_next_instruction_name(), ins=[], outs=[]
                        )
                        nop.engine = inst.engine
                        nop.sync_info = mybir.SyncInfo(on_wait=[w], on_update=[])
                        nop.bass_nofuse = True
                        nc.register_instruction(nop)
                        out.append(nop)
                    si.on_wait = [waits[-1]]
                out.append(inst)
            bb.instructions[:] = out


def _build_program():
    f32 = mybir.dt.float32
    bf16 = mybir.dt.bfloat16
    nc = bass.Bass()
    x_d = nc.declare_dram_parameter("x", [D, S], bf16, isOutput=False)
    # wg packs all small bf16 constants: W1a | W1b | W2 | gf_rep
    WGW = 2 * H + M + 512
    wg_d = nc.declare_dram_parameter("wg", [128, WGW], bf16, isOutput=False)
    o_d = nc.declare_dram_parameter("o", [M, S], bf16, isOutput=True)

    relu = mybir.ActivationFunctionType.Relu

    with TileContext(nc) as tc:
        with (
            tc.tile_pool(name="consts", bufs=1) as cpool,
            tc.tile_pool(name="x", bufs=NT // 2) as xpool,
            tc.tile_pool(name="h", bufs=3) as hpool,
            tc.tile_pool(name="ps", bufs=3, space="PSUM") as pspool,
            tc.tile_pool(name="pswarm", bufs=1, space="PSUM") as pswpool,
            tc.tile_pool(name="ps2", bufs=2, space="PSUM") as ps2pool,
            tc.tile_pool(name="o", bufs=1) as opool,
            tc.tile_pool(name="scratch", bufs=1) as spool,
        ):
            # Warmups, fed by an on-chip memset (no DMA dependency): a
            # [128,1] relu so ACT's PWP table loads during the DMA wait, and
            # two short bf16 matmuls that keep the PE's HAM activity window
            # busy during the x DMA wait (cheap — unlike f32 warmups, which
            # stream at 1/4 rate and monopolize the PE for ~4us).
            warm = spool.tile([128, 512], bf16)
            nc.vector.memset(warm[:], 0.0)
            nc.scalar.activation(warm[:, 0:1], warm[:, 0:1], relu)
            pswarm = pswpool.tile([128, 512], f32)
            for _ in range(2):
                nc.tensor.matmul(
                    pswarm[:], warm[:, 0:128], warm[:], start=True, stop=True
                )

            wg_sb = cpool.tile([128, WGW], bf16)
            nc.sync.dma_start(out=wg_sb[:], in_=wg_d[:])
            w1a_sb = wg_sb[:, 0:H]
            w1b_sb = wg_sb[:, H:2 * H]
            w2_sb = wg_sb[:, 2 * H:2 * H + M]
            gfr_sb = wg_sb[:, 2 * H + M:WGW]

            xts = [
                xpool.tile([D, 1024], bf16, name=f"xt{i}", tag=f"x{i}")
                for i in range(NT // 2)
            ]
            for c in range(NT // 2):
                nc.sync.dma_start(
                    out=xts[c][:], in_=x_d[:, c * 1024:(c + 1) * 1024]
                )

            o_sb = opool.tile([M, S], bf16)

            for t in range(NT):
                xt = xts[t // 2][:, (t % 2) * 512:(t % 2 + 1) * 512]
                ps = pspool.tile([128, 512], f32)
                nc.tensor.matmul(ps[:], w1a_sb, xt, start=True, stop=False)
                nc.tensor.matmul(ps[:], w1b_sb, gfr_sb, start=False, stop=True)
                h = hpool.tile([128, 512], bf16)
                nc.scalar.activation(h[:], ps[:], relu)
                ps2 = ps2pool.tile([M, 512], f32)
                nc.tensor.matmul(ps2[:], w2_sb, h[:], start=True, stop=True)
                # raw bf16 scores out; masking to exact NEG happens host-side
                nc.vector.tensor_copy(
                    out=o_sb[:, t * 512:(t + 1) * 512], in_=ps2[:]
                )
                if t == NT // 2 - 1:
                    nc.gpsimd.dma_start(
                        out=o_d[:, :S // 2], in_=o_sb[:, :S // 2]
                    )
            nc.gpsimd.dma_start(out=o_d[:, S // 2:], in_=o_sb[:, S // 2:])
    _legalize_single_wait(nc)
    return nc


_NC_CACHE = None


def _get_program():
    global _NC_CACHE
    if _NC_CACHE is None:
        _NC_CACHE = _build_program()
    return _NC_CACHE


def _prepare_inputs(node_features, global_features, W1, b1, W2, b2, cube_mask,
                    batch, move_mask):
    """Host-side shard prep. Returns per-core input dicts."""
    node_features = np.asarray(node_features, dtype=np.float32)
    global_features = np.asarray(global_features, dtype=np.float32)
    W1 = np.asarray(W1, dtype=np.float32)
    b1 = np.asarray(b1, dtype=np.float32)
    W2 = np.asarray(W2, dtype=np.float32)
    b2 = np.asarray(b2, dtype=np.float32)
    cube_mask = np.asarray(cube_mask).astype(bool)
    batch = np.asarray(batch).astype(np.int64)
    move_mask = np.asarray(move_mask).astype(bool)
    assert np.all(b1 == 0.0) and np.all(b2 == 0.0), (
        "kernel bakes b1==b2==0 into the min-cap masking"
    )

    # First-64 cube nodes per graph (matches the reference's cube_idx math).
    idx = np.flatnonzero(cube_mask)                     # cube nodes, node order
    cb = batch[idx]                                     # their graph (sorted)
    counts = np.bincount(cb, minlength=B)
    starts = np.concatenate([[0], np.cumsum(counts)[:-1]])
    pos = np.arange(idx.shape[0], dtype=np.int64) - starts[cb]
    sel = pos < MC
    vidx, vb, vpos = idx[sel], cb[sel], pos[sel]

    gather_idx = np.zeros((B, MC), dtype=np.int64)
    valid = np.zeros((B, MC), dtype=bool)
    gather_idx[vb, vpos] = vidx
    valid[vb, vpos] = True

    wcat = np.concatenate([W1[:D], W1[D:], W2], axis=1)  # [128, 2H + M]

    in_maps = []
    oks = []
    for k in range(NCORES):
        gb = slice(k * GPC, (k + 1) * GPC)
        gi = gather_idx[gb]                             # [GPC, MC]
        # slot-major: node j = c*GPC + g  ->  (cube slot c, graph g)
        order = gi.T.reshape(-1)                        # [S]
        x = np.ascontiguousarray(
            node_features[order].T.astype(ml_dtypes.bfloat16)
        )                                               # [D, S]
        gfr = np.tile(global_features[gb].T, (1, 512 // GPC))    # [G, 512]
        wg = np.ascontiguousarray(
            np.concatenate([wcat, gfr], axis=1).astype(ml_dtypes.bfloat16)
        )                                               # [128, 2H + M + 512]
        ok = valid[gb][:, :, None] & move_mask[gb]      # [GPC, MC, M]
        oks.append(ok)
        in_maps.append({"x": x, "wg": wg})
    return in_maps, oks


def _decode_outputs(results, oks):
    logits = np.empty((B, MC, M), dtype=np.float32)
    for k in range(NCORES):
        o = np.asarray(results[k]["o"]).astype(np.float32)   # [M, S]
        # slot-major: column j = c*GPC + g
        scores = o.reshape(M, MC, GPC).transpose(2, 1, 0)    # [GPC, MC, M]
        logits[k * GPC:(k + 1) * GPC] = np.where(
            oks[k], scores, np.float32(NEG)
        )
    return logits.reshape(B, MC * M)


def kernel(**inputs) -> np.ndarray:
    in_maps, oks = _prepare_inputs(**inputs)
    nc = _get_program()
    res = run_bass_kernel_spmd(nc, in_maps, list(range(NCORES)))
    return _decode_outputs(res.results, oks)


# revision 10
# speedup vs baseline: 1.1427x; 1.0559x over previous
"""Trainium2 Bass kernel for nn_CubeMoveHead.

Contract: kernel(**inputs) takes the FULL unsharded inputs (as produced by
setup_inputs) and returns the FULL [512, 1536] float32 output.

Strategy (data-parallel over graphs, 64 graphs per core on 8 cores):
  Only the first 64 cube nodes of each graph ever reach the output, so the
  host computes those node indices (pure index math on cube_mask/batch),
  gathers just the needed node_features rows (4096 per core), transposes
  them to the matmul-friendly [D, nodes] layout, and ships them to each
  core's HBM in bf16. Nodes are laid out slot-major (node j on a core is
  cube slot c = j // 64 of graph g = j % 64), so the per-graph global
  feature column tiles periodically: gf_rep[:, j] = gf[j % 64].

  All matmul inputs are bf16 (f32 PSUM accumulate): measured end-to-end
  rel err ~4e-3 against the f32 reference, well inside the 2e-2 gate.

  On-device per core, for each of 8 tiles of 512 node slots:
    ps  = W1a.T @ x_t + W1b.T @ gf_rep   (two accumulating matmuls, PSUM)
    h   = relu(ps)  -> bf16              (ACT, PSUM->SBUF)
    ps2 = W2.T @ h                       ([24, 512] PSUM; W2 stationary so
                                          the whole tile streams in one
                                          512-col matmul instead of 4
                                          LDWEIGHTS+24-col matmuls)
    o   = min(ps2, cap)                  (DVE; cap = +BIG where slot valid
                                          & move allowed, else NEG)
  min-cap masking yields exactly NEG on masked positions; it folds b1/b2,
  which are identically zero in the reference (asserted host-side).

  The first matmuls start right after the small weight DMA lands and run
  back-to-back with no long PE gaps, so the HAM clock gate warms to 2.4
  GHz early and stays there (the previous version stalled the PE ~4us
  waiting for a late DMA and ran the whole kernel re-throttled at 1.2).
"""

import sys

if "/opt/trn_rl_repo" not in sys.path:
    sys.path.insert(0, "/opt/trn_rl_repo")

import ml_dtypes
import numpy as np

import concourse.bass as bass
import concourse.mybir as mybir
from concourse.tile import TileContext
from concourse.bass_utils import run_bass_kernel_spmd

N = 500000
B = 512
D = 128
G = 128
MC = 64
M = 24
H = 128
NEG = -1.0e9
BIG = 3.0e38
NCORES = 8
GPC = B // NCORES          # graphs per core (64)
S = GPC * MC               # node slots per core (4096)
NT = S // 512              # 512-slot tiles per core (8)


def _legalize_single_wait(nc):
    """The walrus build here accepts at most ONE sync wait per instruction;
    Tile's scheduler happily emits several. Hoist extra waits onto same-engine
    nops inserted immediately before the offending instruction (same engine
    executes in order, so the happens-before is preserved exactly)."""
    for f in nc.m.functions:
        for bb in f.blocks:
            insts = bb.instructions
            if not any(
                i.sync_info and i.sync_info.on_wait and len(i.sync_info.on_wait) > 1
                for i in insts
            ):
                continue
            out = []
            for inst in insts:
                si = inst.sync_info
                waits = list(si.on_wait) if si and si.on_wait else []
                if len(waits) > 1:
                    for w in waits[:-1]:
                        nop = mybir.InstNoOp(
                            name=nc.get_next_instruction_name(), ins=[], outs=[]
                        )
                        nop.engine = inst.engine
                        nop.sync_info = mybir.SyncInfo(on_wait=[w], on_update=[])
                        nop.bass_nofuse = True
                        nc.register_instruction(nop)
                        out.append(nop)
                    si.on_wait = [waits[-1]]
                out.append(inst)
            bb.instructions[:] = out


def _build_program():
    f32 = mybir.dt.float32
    bf16 = mybir.dt.bfloat16
    nc = bass.Bass()
    x_d = nc.declare_dram_parameter("x", [D, S], bf16, isOutput=False)
    # wg packs all small bf16 constants: W1a | W1b | W2 | gf (unreplicated)
    WGW = 2 * H + M + GPC
    wg_d = nc.declare_dram_parameter("wg", [128, WGW], bf16, isOutput=False)
    o_d = nc.declare_dram_parameter("o", [M, S], bf16, isOutput=True)

    relu = mybir.ActivationFunctionType.Relu

    with TileContext(nc) as tc:
        with (
            tc.tile_pool(name="consts", bufs=1) as cpool,
            tc.tile_pool(name="x", bufs=NT // 2) as xpool,
            tc.tile_pool(name="h", bufs=3) as hpool,
            tc.tile_pool(name="ps", bufs=3, space="PSUM") as pspool,
            tc.tile_pool(name="pswarm", bufs=1, space="PSUM") as pswpool,
            tc.tile_pool(name="ps2", bufs=2, space="PSUM") as ps2pool,
            tc.tile_pool(name="o", bufs=1) as opool,
            tc.tile_pool(name="scratch", bufs=1) as spool,
        ):
            # Warmups, fed by an on-chip memset (no DMA dependency): a
            # [128,1] relu so ACT's PWP table loads during the DMA wait, and
            # two short bf16 matmuls that keep the PE's HAM activity window
            # busy during the x DMA wait (cheap — unlike f32 warmups, which
            # stream at 1/4 rate and monopolize the PE for ~4us).
            warm = spool.tile([128, 512], bf16)
            nc.vector.memset(warm[:], 0.0)
            nc.scalar.activation(warm[:, 0:1], warm[:, 0:1], relu)
            pswarm = pswpool.tile([128, 512], f32)
            nc.tensor.matmul(
                pswarm[:], warm[:, 0:128], warm[:], start=True, stop=True
            )

            wg_sb = cpool.tile([128, WGW], bf16)
            nc.sync.dma_start(out=wg_sb[:], in_=wg_d[:])
            w1a_sb = wg_sb[:, 0:H]
            w1b_sb = wg_sb[:, H:2 * H]
            w2_sb = wg_sb[:, 2 * H:2 * H + M]
            # gf broadcast: read the [128, 64] block 8x via a stride-0 dim
            gfr_b = wg_sb[:, None, 2 * H + M:WGW].broadcast_to([128, NT, GPC])

            xts = [
                xpool.tile([D, 512], bf16, name=f"xt{i}", tag=f"x{i}")
                for i in range(NT)
            ]
            for c in range(NT):
                nc.sync.dma_start(
                    out=xts[c][:], in_=x_d[:, c * 512:(c + 1) * 512]
                )

            o_sb = opool.tile([M, S], bf16)

            for t in range(NT):
                ps = pspool.tile([128, 512], f32)
                nc.tensor.matmul(ps[:], w1a_sb, xts[t][:], start=True, stop=False)
                nc.tensor.matmul(ps[:], w1b_sb, gfr_b, start=False, stop=True)
                h = hpool.tile([128, 512], bf16)
                nc.scalar.activation(h[:], ps[:], relu)
                ps2 = ps2pool.tile([M, 512], f32)
                nc.tensor.matmul(ps2[:], w2_sb, h[:], start=True, stop=True)
                # raw bf16 scores out; masking to exact NEG happens host-side
                nc.vector.tensor_copy(
                    out=o_sb[:, t * 512:(t + 1) * 512], in_=ps2[:]
                )
                if t % 2 == 1:
                    nc.scalar.dma_start(
                        out=o_d[:, (t - 1) * 512:(t + 1) * 512],
                        in_=o_sb[:, (t - 1) * 512:(t + 1) * 512],
                    )
    _legalize_single_wait(nc)
    return nc


_NC_CACHE = None


def _get_program():
    global _NC_CACHE
    if _NC_CACHE is None:
        _NC_CACHE = _build_program()
    return _NC_CACHE


def _prepare_inputs(node_features, global_features, W1, b1, W2, b2, cube_mask,
                    batch, move_mask):
    """Host-side shard prep. Returns per-core input dicts."""
    node_features = np.asarray(node_features, dtype=np.float32)
    global_features = np.asarray(global_features, dtype=np.float32)
    W1 = np.asarray(W1, dtype=np.float32)
    b1 = np.asarray(b1, dtype=np.float32)
    W2 = np.asarray(W2, dtype=np.float32)
    b2 = np.asarray(b2, dtype=np.float32)
    cube_mask = np.asarray(cube_mask).astype(bool)
    batch = np.asarray(batch).astype(np.int64)
    move_mask = np.asarray(move_mask).astype(bool)
    assert np.all(b1 == 0.0) and np.all(b2 == 0.0), (
        "kernel bakes b1==b2==0 into the min-cap masking"
    )

    # First-64 cube nodes per graph (matches the reference's cube_idx math).
    idx = np.flatnonzero(cube_mask)                     # cube nodes, node order
    cb = batch[idx]                                     # their graph (sorted)
    counts = np.bincount(cb, minlength=B)
    starts = np.concatenate([[0], np.cumsum(counts)[:-1]])
    pos = np.arange(idx.shape[0], dtype=np.int64) - starts[cb]
    sel = pos < MC
    vidx, vb, vpos = idx[sel], cb[sel], pos[sel]

    gather_idx = np.zeros((B, MC), dtype=np.int64)
    valid = np.zeros((B, MC), dtype=bool)
    gather_idx[vb, vpos] = vidx
    valid[vb, vpos] = True

    wcat = np.concatenate([W1[:D], W1[D:], W2], axis=1)  # [128, 2H + M]

    in_maps = []
    oks = []
    for k in range(NCORES):
        gb = slice(k * GPC, (k + 1) * GPC)
        gi = gather_idx[gb]                             # [GPC, MC]
        # slot-major: node j = c*GPC + g  ->  (cube slot c, graph g)
        order = gi.T.reshape(-1)                        # [S]
        x = np.ascontiguousarray(
            node_features[order].T.astype(ml_dtypes.bfloat16)
        )                                               # [D, S]
        wg = np.ascontiguousarray(
            np.concatenate([wcat, global_features[gb].T], axis=1)
            .astype(ml_dtypes.bfloat16)
        )                                               # [128, 2H + M + GPC]
        ok = valid[gb][:, :, None] & move_mask[gb]      # [GPC, MC, M]
        oks.append(ok)
        in_maps.append({"x": x, "wg": wg})
    return in_maps, oks


def _decode_outputs(results, oks):
    logits = np.empty((B, MC, M), dtype=np.float32)
    for k in range(NCORES):
        o = np.asarray(results[k]["o"]).astype(np.float32)   # [M, S]
        # slot-major: column j = c*GPC + g
        scores = o.reshape(M, MC, GPC).transpose(2, 1, 0)    # [GPC, MC, M]
        logits[k * GPC:(k + 1) * GPC] = np.where(
            oks[k], scores, np.float32(NEG)
        )
    return logits.reshape(B, MC * M)


def kernel(**inputs) -> np.ndarray:
    in_maps, oks = _prepare_inputs(**inputs)
    nc = _get_program()
    res = run_bass_kernel_spmd(nc, in_maps, list(range(NCORES)))
    return _decode_outputs(res.results, oks)
